# revision 23
# baseline (speedup 1.0000x reference)
"""Butterfly transform kernel for Trainium2 (8 NeuronCores, SPMD data parallel).

Math: the reference applies 12 butterfly layers; every layer pairs the SAME
adjacent columns (2n, 2n+1) and multiplies each pair by a per-pair 2x2
matrix W[l, n].  The composition collapses into a single per-pair 2x2 matrix
M[n] = W[0,n] @ ... @ W[11,n], so the device kernel is ONE memory-bound pass:

    y[:, 2n]   = x[:, 2n] * M[n,0,0] + x[:, 2n+1] * M[n,1,0]
    y[:, 2n+1] = x[:, 2n] * M[n,0,1] + x[:, 2n+1] * M[n,1,1]

i.e.  y = x*A + pairswap(x*B)  with full-width interleaved coefficient
vectors A (diag terms) and B (cross terms).

Device layout: batch rows on SBUF partitions, features along the free dim.
HBM traffic is fixed at 32 MiB/core (16 in + 16 out, f32) ~= 93.7 us at the
~358 GB/s per-core HBM limit.  In f32 the three DVE tensor_tensor passes run
at 1x mode (~106 us) and bound the kernel (~122 us steady-state measured).
The shipped mode ("v4qs") moves the arithmetic to bf16 so the DVE runs its
2x perf modes and the kernel becomes DMA-bound (~98 us steady-state;
tolerance is rel-err < 2e-2, bf16 contributes ~3e-3):

  - load:  one SWDGE dma per [128, 4096] tile casts HBM f32 -> SBUF bf16
           (nc.gpsimd dma does dtype conversion inline); halves the
           SBUF-side DMA bytes and leaves all HWDGE queues to the stores
  - DVE:   yb = xb*A, vb = xb*B (bf16 tensor_tensor, 2x_1P packed mode,
           ~2.2us/tile each), then y = yb + pairswap(vb) as two
           half-width adds (bf16 in, f32 out, the swapped operand's
           misaligned access pattern drops these to 1x - still far under
           the DMA pace)
  - store: each half-add's f32 result is stored by its own 1 MiB HWDGE
           dma as soon as that half finishes (2 queues/tile in flight;
           measured ~2us/pass faster than one 2 MiB store)

Measured steady-state (For_i-looped pass, differential): this kernel
~97-98.5 us vs 122 us for the f32 3-op version and ~101.7 us for a pure
f32 copy probe -- ~97% of the 93.7 us HBM roofline.  Exhausted axes (all
within +-2 us noise or worse): mega 4 MiB DMAs (equal), [128, 2048]
sub-tiles (worse, +4.5 us), 4-way store split (worse), load splitting
(worse), all-SWDGE patterns (+5 us, Q7 descriptor rings contend with DVE
2-port packed ops), single_packet (equal), num_swdge_queues 1/2 (equal),
staggered_reset vs barrier back-edge (equal => no hidden per-pass
ramp/tail).  The folded weights are replicated across the 128 partitions
at startup by a ones-matmul on the otherwise-idle PE, cast to bf16 on the
PSUM->SBUF copy; the weight chain hides entirely under the DMA stream
(compute has ~30 us of slack).
"""

import sys
import numpy as np

if "/opt/trn_rl_repo" not in sys.path:
    sys.path.insert(0, "/opt/trn_rl_repo")

BATCH = 8192
SIZE = 4096
LOG_N = 12
HALF = SIZE // 2  # 2048
N_CORES = 8
ROWS_PER_CORE = BATCH // N_CORES  # 1024
P = 128  # SBUF partitions
N_TILES = ROWS_PER_CORE // P  # 8

_CACHE = {}


def _build_program(mode: str = "v4qs", loop_n: int = 1, xio_bufs: int = 6, swq: int = 4, stagger: bool = False):
    import concourse.bass as bass
    import concourse.bacc as bacc
    import concourse.mybir as mybir
    from concourse import tile
    from contextlib import ExitStack

    f32 = mybir.dt.float32
    bf16 = mybir.dt.bfloat16
    nc = bacc.Bacc(None, num_swdge_queues=swq)

    x_in = nc.dram_tensor("x", [ROWS_PER_CORE, SIZE], f32, kind="ExternalInput")
    w_in = nc.dram_tensor("wf", [1, 2 * SIZE], f32, kind="ExternalInput")
    y_out = nc.dram_tensor("y", [ROWS_PER_CORE, SIZE], f32, kind="ExternalOutput")

    mult = mybir.AluOpType.mult
    add = mybir.AluOpType.add

    bf_mode = mode in ("v1", "v2", "v4", "v4q", "v4q2", "v4q4", "v4qs", "v4sp", "v12", "v13", "v13b", "v8", "copycast", "copybig")
    probe = mode in ("copy", "copycast", "copybig", "copybigh")
    mega = mode in ("copybig", "copybigh")
    wdt = bf16 if bf_mode else f32

    with tile.TileContext(nc) as tc, ExitStack() as ctx:
        xio = ctx.enter_context(
            tc.tile_pool(name="xio", bufs=xio_bufs if bf_mode else 4)
        )
        if not probe:
            const = ctx.enter_context(tc.tile_pool(name="const", bufs=1))
            yio = ctx.enter_context(tc.tile_pool(name="yio", bufs=2 if mode == "v8" else 3))
            t1 = ctx.enter_context(tc.tile_pool(name="t1", bufs=3))
            if mode in ("v1", "v8"):
                t2 = ctx.enter_context(tc.tile_pool(name="t2", bufs=3))
                t3 = ctx.enter_context(tc.tile_pool(name="t3", bufs=3))
            elif mode in ("v2", "v4", "v4q", "v4q2", "v4q4", "v4qs", "v4sp", "v12", "v13", "v13b", "v8"):
                t2 = ctx.enter_context(tc.tile_pool(name="t2", bufs=3))
            psum = ctx.enter_context(tc.tile_pool(name="psum", bufs=2, space="PSUM"))

            # Broadcast the 2 interleaved full-width weight vectors to all
            # 128 partitions: one DMA brings the row into partition 0 (cast
            # to bf16 on the way for the bf16 modes), a ones-matmul on the
            # idle PE replicates it across partitions, and the ACT engine
            # copies PSUM->SBUF (casting back down for bf16).
            ones = const.tile([1, P], f32)
            nc.vector.memset(ones[:], 1.0)
            wb = const.tile([P, 2 * SIZE], wdt)
            if bf_mode:
                # f32 row via HWDGE (keeps the SWDGE queues clear for the
                # first x loads); the PSUM->SBUF copy casts to bf16.
                wrow_t = const.tile([1, 2 * SIZE], f32)
                wrow = wrow_t[0:1, :]
            else:
                wrow = wb[0:1, :]
            nc.sync.dma_start(wrow, w_in[:])
            for c in range(4):
                pt = psum.tile([P, HALF], f32, tag="wpsum")
                for j in range(HALF // 512):
                    nc.tensor.matmul(
                        pt[:, j * 512 : (j + 1) * 512],
                        ones[:],
                        wrow[:, c * HALF + j * 512 : c * HALF + (j + 1) * 512],
                        start=True,
                        stop=True,
                    )
                nc.scalar.copy(wb[:, c * HALF : (c + 1) * HALF], pt[:])
            a_full = wb[:, 0:SIZE]   # [A00 A11 A00 A11 ...] per pair
            b_full = wb[:, SIZE:]    # [A01 A10 A01 A10 ...] per pair

        def tile_body(i):
            rows = slice(i * P, (i + 1) * P)
            if mode == "copy":
                # DMA roofline probe: f32 load + f32 store, no compute.
                xt = xio.tile([P, SIZE], f32, tag="xt")
                nc.sync.dma_start(xt[:], x_in[rows, :])
                nc.gpsimd.dma_start(y_out[rows, :], xt[:])
                return
            if mode == "copycast":
                # Cast-DMA roofline probe: SWDGE f32->bf16 load, bf16->f32
                # store. Same HBM bytes as `copy`, half the SBUF bytes.
                xb = xio.tile([P, SIZE], bf16, tag="xt")
                nc.gpsimd.dma_start(xb[:], x_in[rows, :])
                nc.gpsimd.dma_start(y_out[rows, :], xb[:])
                return
            if mode == "swst":
                # f32 baseline: 3 DVE tensor_tensor passes at 1x mode.
                xt = xio.tile([P, SIZE], f32, tag="xt")
                nc.sync.dma_start(xt[:], x_in[rows, :])
                yt = yio.tile([P, SIZE], f32, tag="yt")
                nc.vector.tensor_tensor(yt[:], xt[:], a_full, mult)
                vt = t1.tile([P, SIZE], f32, tag="vt")
                nc.vector.tensor_tensor(vt[:], xt[:], b_full, mult)
                y3 = yt[:].rearrange("p (n two) -> p n two", two=2)
                v3 = vt[:].rearrange("p (n two) -> p n two", two=2)[:, :, ::-1]
                nc.vector.tensor_tensor(y3, y3, v3, add)
                nc.gpsimd.dma_start(y_out[rows, :], yt[:])
                return
            if mode == "v1":
                # bf16 everywhere; swap on ACT; cast DMAs both ways.
                xb = xio.tile([P, SIZE], bf16, tag="xt")
                nc.gpsimd.dma_start(xb[:], x_in[rows, :])
                yb = t1.tile([P, SIZE], bf16, tag="yb")
                nc.vector.tensor_tensor(yb[:], xb[:], a_full, mult)
                vb = t2.tile([P, SIZE], bf16, tag="vb")
                nc.vector.tensor_tensor(vb[:], xb[:], b_full, mult)
                sw = t3.tile([P, SIZE], bf16, tag="sw")
                v3 = vb[:].rearrange("p (n two) -> p n two", two=2)[:, :, ::-1]
                s3 = sw[:].rearrange("p (n two) -> p n two", two=2)
                nc.scalar.copy(s3, v3)
                yo = yio.tile([P, SIZE], bf16, tag="yt")
                nc.vector.tensor_tensor(yo[:], yb[:], sw[:], add)
                nc.gpsimd.dma_start(y_out[rows, :], yo[:])
                return
            if mode == "v2":
                # bf16 muls; swap-add on DVE (1x) straight to f32 out;
                # cast load, plain f32 store on SWDGE.
                xb = xio.tile([P, SIZE], bf16, tag="xt")
                nc.gpsimd.dma_start(xb[:], x_in[rows, :])
                yb = t1.tile([P, SIZE], bf16, tag="yb")
                nc.vector.tensor_tensor(yb[:], xb[:], a_full, mult)
                vb = t2.tile([P, SIZE], bf16, tag="vb")
                nc.vector.tensor_tensor(vb[:], xb[:], b_full, mult)
                yt = yio.tile([P, SIZE], f32, tag="yt")
                y3 = yt[:].rearrange("p (n two) -> p n two", two=2)
                yb3 = yb[:].rearrange("p (n two) -> p n two", two=2)
                v3 = vb[:].rearrange("p (n two) -> p n two", two=2)[:, :, ::-1]
                nc.vector.tensor_tensor(y3, yb3, v3, add)
                nc.gpsimd.dma_start(y_out[rows, :], yt[:])
                return
            if mode == "v8":
                # balanced stores: even tiles store f32 via HWDGE (split 2),
                # odd tiles store bf16-cast via SWDGE.
                xb = xio.tile([P, SIZE], bf16, tag="xt")
                nc.gpsimd.dma_start(xb[:], x_in[rows, :])
                yb = t1.tile([P, SIZE], bf16, tag="yb")
                nc.vector.tensor_tensor(yb[:], xb[:], a_full, mult)
                vb = t2.tile([P, SIZE], bf16, tag="vb")
                nc.vector.tensor_tensor(vb[:], xb[:], b_full, mult)
                odt = f32 if i % 2 == 0 else bf16
                pool = yio if odt == f32 else t3
                yt = pool.tile([P, SIZE], odt, tag="yt" + ("f" if odt == f32 else "b"))
                y3 = yt[:].rearrange("p (n two) -> p n two", two=2)
                yb3 = yb[:].rearrange("p (n two) -> p n two", two=2)
                v3 = vb[:].rearrange("p (n two) -> p n two", two=2)[:, :, ::-1]
                nc.vector.tensor_tensor(y3, yb3, v3, add)
                if odt == f32:
                    nc.sync.dma_start(y_out[rows, 0:HALF], yt[:, :HALF])
                    nc.sync.dma_start(y_out[rows, HALF:], yt[:, HALF:])
                else:
                    nc.gpsimd.dma_start(y_out[rows, :], yt[:])
                return
            if mode == "v12":
                # finer granularity: two [128, 2048] column sub-tiles per
                # row block; 16 x 1 MiB cast loads + 16 x 1 MiB f32 stores.
                for h in range(2):
                    cols = slice(h * HALF, (h + 1) * HALF)
                    xb = xio.tile([P, HALF], bf16, tag="xt")
                    nc.gpsimd.dma_start(xb[:], x_in[rows, cols])
                    yb = t1.tile([P, HALF], bf16, tag="yb")
                    nc.vector.tensor_tensor(yb[:], xb[:], a_full[:, cols], mult)
                    vb = t2.tile([P, HALF], bf16, tag="vb")
                    nc.vector.tensor_tensor(vb[:], xb[:], b_full[:, cols], mult)
                    yt = yio.tile([P, HALF], f32, tag="yt")
                    y3 = yt[:].rearrange("p (n two) -> p n two", two=2)
                    yb3 = yb[:].rearrange("p (n two) -> p n two", two=2)
                    v3 = vb[:].rearrange("p (n two) -> p n two", two=2)[
                        :, :, ::-1
                    ]
                    nc.vector.tensor_tensor(y3, yb3, v3, add)
                    nc.sync.dma_start(y_out[rows, cols], yt[:])
                return
            if mode in ("v4qs", "v4sp", "v13", "v13b"):
                spk = mode == "v4sp"
                xb = xio.tile([P, SIZE], bf16, tag="xt")
                nc.gpsimd.dma_start(xb[:], x_in[rows, :], single_packet=spk)
                yb = t1.tile([P, SIZE], bf16, tag="yb")
                nc.vector.tensor_tensor(yb[:], xb[:], a_full, mult)
                vb = t2.tile([P, SIZE], bf16, tag="vb")
                nc.vector.tensor_tensor(vb[:], xb[:], b_full, mult)
                yt = yio.tile([P, SIZE], f32, tag="yt")
                for h in range(2):
                    cols = slice(h * HALF, (h + 1) * HALF)
                    y3 = yt[:, cols].rearrange("p (n two) -> p n two", two=2)
                    yb3 = yb[:, cols].rearrange("p (n two) -> p n two", two=2)
                    v3 = vb[:, cols].rearrange("p (n two) -> p n two", two=2)[
                        :, :, ::-1
                    ]
                    nc.vector.tensor_tensor(y3, yb3, v3, add)
                    # v13: spread the half-stores over BOTH physical HWDGE
                    # rings (SP via nc.sync, ACT via nc.scalar - idle here).
                    # v13b: alternate rings per tile instead of per half.
                    if mode == "v13":
                        eng = nc.sync if h == 0 else nc.scalar
                    elif mode == "v13b":
                        eng = nc.sync if i % 2 == 0 else nc.scalar
                    else:
                        eng = nc.sync
                    eng.dma_start(
                        y_out[rows, cols], yt[:, cols], single_packet=spk
                    )
                return
            if mode in ("v4", "v4q", "v4q2", "v4q4"):
                # like v2 but f32 store on HWDGE (frees SWDGE for loads).
                # v4q*: stores (and for v4q2 loads) split into multiple
                # DMAs for queue concurrency.
                xb = xio.tile([P, SIZE], bf16, tag="xt")
                if mode == "v4q2":
                    nc.gpsimd.dma_start(xb[:, :HALF], x_in[rows, 0:HALF])
                    nc.gpsimd.dma_start(xb[:, HALF:], x_in[rows, HALF:])
                else:
                    nc.gpsimd.dma_start(xb[:], x_in[rows, :])
                yb = t1.tile([P, SIZE], bf16, tag="yb")
                nc.vector.tensor_tensor(yb[:], xb[:], a_full, mult)
                vb = t2.tile([P, SIZE], bf16, tag="vb")
                nc.vector.tensor_tensor(vb[:], xb[:], b_full, mult)
                yt = yio.tile([P, SIZE], f32, tag="yt")
                y3 = yt[:].rearrange("p (n two) -> p n two", two=2)
                yb3 = yb[:].rearrange("p (n two) -> p n two", two=2)
                v3 = vb[:].rearrange("p (n two) -> p n two", two=2)[:, :, ::-1]
                nc.vector.tensor_tensor(y3, yb3, v3, add)
                if mode == "v4q4":
                    Q = SIZE // 4
                    for q in range(4):
                        nc.sync.dma_start(
                            y_out[rows, q * Q : (q + 1) * Q],
                            yt[:, q * Q : (q + 1) * Q],
                        )
                elif mode in ("v4q", "v4q2"):
                    nc.sync.dma_start(y_out[rows, 0:HALF], yt[:, :HALF])
                    nc.sync.dma_start(y_out[rows, HALF:], yt[:, HALF:])
                else:
                    nc.sync.dma_start(y_out[rows, :], yt[:])
                return
            raise ValueError(mode)

        def mega_body(i):
            # Probe: fused 2-row-block (4 MiB HBM-side) DMAs, no compute.
            xv = x_in[:].rearrange("(t a p) n -> t p a n", p=P, a=2)
            yv = y_out[:].rearrange("(t a p) n -> t p a n", p=P, a=2)
            if mode == "copybig":
                # all-SWDGE cast, 4 MiB transfers (copycast w/ big DMAs)
                xb = xio.tile([P, 2 * SIZE], bf16, tag="xt")
                x3 = xb[:].rearrange("p (a n) -> p a n", a=2)
                nc.gpsimd.dma_start(x3, xv[i])
                nc.gpsimd.dma_start(yv[i], x3)
            elif mode == "copybigh":
                # HWDGE f32 loads + SWDGE f32 stores, 4 MiB transfers
                xt = xio.tile([P, 2 * SIZE], f32, tag="xt")
                x3 = xt[:].rearrange("p (a n) -> p a n", a=2)
                nc.sync.dma_start(x3, xv[i])
                nc.gpsimd.dma_start(yv[i], x3)
            else:
                raise ValueError(mode)

        def pass_body():
            if mega:
                for i in range(N_TILES // 2):
                    mega_body(i)
            else:
                for i in range(N_TILES):
                    tile_body(i)

        if loop_n > 1:
            with tc.For_i(0, loop_n, 1, staggered_reset=stagger):
                pass_body()
        else:
            pass_body()

    nc.compile()
    return nc


def _get_nc(
    mode: str = "v4qs",
    loop_n: int = 1,
    xio_bufs: int = 6,
    swq: int = 4,
    stagger: bool = False,
):
    key = ("nc", mode, loop_n, xio_bufs, swq, stagger)
    if key not in _CACHE:
        _CACHE[key] = _build_program(mode, loop_n, xio_bufs, swq, stagger)
    return _CACHE[key]


def fold_weights(W: np.ndarray) -> np.ndarray:
    """Compose the 12 stacked per-pair 2x2 layers into one, in float64.

    Returns wf [1, 2*SIZE] float32: full-width A (diag: A00,A11 interleaved)
    followed by full-width B (cross: A01,A10 interleaved)."""
    Wd = W.astype(np.float64)  # [12, HALF, 2, 2]
    M = Wd[0]
    for l in range(1, Wd.shape[0]):
        M = np.einsum("nij,njk->nik", M, Wd[l])
    M = M.astype(np.float32)  # [HALF, 2, 2]
    a_full = np.stack([M[:, 0, 0], M[:, 1, 1]], axis=1).reshape(SIZE)
    b_full = np.stack([M[:, 0, 1], M[:, 1, 0]], axis=1).reshape(SIZE)
    wf = np.concatenate([a_full, b_full])
    return np.ascontiguousarray(wf.reshape(1, 2 * SIZE))


def _run(x: np.ndarray, W: np.ndarray, mode: str = "v4qs", **run_kwargs):
    """Shard, run on the 8 cores, gather. Returns (output, BassKernelResults)."""
    import time

    from concourse.bass_utils import run_bass_kernel_spmd

    assert x.shape == (BATCH, SIZE) and W.shape == (LOG_N, HALF, 2, 2)
    x = np.ascontiguousarray(x, dtype=np.float32)
    wf = fold_weights(np.asarray(W))

    nc = _get_nc(mode)
    in_maps = [
        {"x": x[c * ROWS_PER_CORE : (c + 1) * ROWS_PER_CORE], "wf": wf}
        for c in range(N_CORES)
    ]
    # The axon-proxied device intermittently dies with
    # NRT_EXEC_UNIT_UNRECOVERABLE (infra flakiness, not kernel-caused —
    # the same NEFF runs clean on retry). Retry a couple of times.
    last_err = None
    for attempt in range(3):
        try:
            res = run_bass_kernel_spmd(
                nc, in_maps, core_ids=list(range(N_CORES)), **run_kwargs
            )
            out = np.concatenate(
                [res.results[c]["y"] for c in range(N_CORES)], axis=0
            )
            return out, res
        except Exception as e:  # noqa: BLE001 - device-level flake, retry
            last_err = e
            if attempt < 2:
                time.sleep(10)
    raise last_err


def kernel(x: np.ndarray, W: np.ndarray) -> np.ndarray:
    return _run(x, W)[0]
